# revision 2
# baseline (speedup 1.0000x reference)
"""GQA kernel for Trainium2, 8 NeuronCores.

Sharding: core c = (b, g) with b = c // 4 (batch), g = c % 4 (KV group).
Each core computes, for its batch b and group g (4 query heads, 1 KV head):
  qT[d, t] for the 4 heads, kT[d, t], v[t, d] projections (contraction over EMB,
  inputs pre-transposed on host so EMB lands on SBUF partitions),
  causal flash-style attention in [k-part, q-free] score layout,
  and the partial output projection  partial_g = (attn out) @ Wp[:, g cols].T.
Host gathers: y[b] = sum_g partial[b, g] + bp.

All matmuls run in bf16 (fp32 PSUM accumulation); host pre-casts inputs.
"""

import numpy as np
import ml_dtypes

T = 2048
EMB = 2048
HD = 128
GS = 4          # query heads per core (per KV group)
NE = EMB // 128 # 16 contraction chunks
NT = T // 128   # 16 row tiles
NQP = T // 512  # 4 q passes of 512
SCALE = float(HD) ** -0.5

_BF16 = ml_dtypes.bfloat16
_PROGRAM = None


def _build_program():
    import concourse.bass as bass
    import concourse.tile as tile
    from concourse import bacc, mybir
    from concourse.masks import make_identity

    f32 = mybir.dt.float32
    bf16 = mybir.dt.bfloat16

    nc = bacc.Bacc("TRN2", target_bir_lowering=False, debug=False)

    xT_d = nc.dram_tensor("xT", [EMB, T], bf16, kind="ExternalInput").rearrange(
        "(c p) t -> c p t", p=128
    )
    wq_d = nc.dram_tensor("wqT", [EMB, GS * HD], bf16, kind="ExternalInput").rearrange(
        "(c p) d -> c p d", p=128
    )
    wk_d = nc.dram_tensor("wkT", [EMB, HD], bf16, kind="ExternalInput").rearrange(
        "(c p) d -> c p d", p=128
    )
    wv_d = nc.dram_tensor("wvT", [EMB, HD], bf16, kind="ExternalInput").rearrange(
        "(c p) d -> c p d", p=128
    )
    wp_d = nc.dram_tensor("wpT", [GS * HD, EMB], bf16, kind="ExternalInput").rearrange(
        "(c p) j -> c p j", p=128
    )
    out_d = nc.dram_tensor("partial", [T, EMB], f32, kind="ExternalOutput").rearrange(
        "(n p) m -> n p m", p=128
    )

    with tile.TileContext(nc) as tc:
        with (
            tc.tile_pool(name="big", bufs=1) as big,
            tc.tile_pool(name="pt", bufs=20) as ptp,
            tc.tile_pool(name="onorm", bufs=4) as onp,
            tc.tile_pool(name="ostage", bufs=4) as osp,
            tc.tile_pool(name="small", bufs=6) as smp,
            tc.tile_pool(name="mm", bufs=4, space="PSUM") as pmm,
            tc.tile_pool(name="oext", bufs=2, space="PSUM") as pox,
            tc.tile_pool(name="tr", bufs=2, space="PSUM") as ptr,
        ):
            xT_sb = big.tile([128, NE * T], bf16)
            wq_sb = big.tile([128, NE * GS * HD], bf16)
            wk_sb = big.tile([128, NE * HD], bf16)
            wv_sb = big.tile([128, NE * HD], bf16)
            wp_sb = big.tile([128, GS * EMB], bf16)
            qT_sb = big.tile([128, GS * T], bf16)
            kT_sb = big.tile([128, T], bf16)
            vext_sb = big.tile([128, NT * (HD + 1)], bf16)
            ohT_sb = big.tile([128, GS * T], bf16)
            ident = big.tile([128, 128], bf16)
            masks = big.tile([128, 4 * 512], bf16)

            # constants: identity for PE transpose, causal masks for the 4
            # diagonal-block offsets (keep iff q_local >= 128*o + k_local)
            make_identity(nc, ident)
            nc.gpsimd.memset(masks, 1.0)
            for o in range(4):
                nc.gpsimd.affine_select(
                    out=masks[:, o * 512 : (o + 1) * 512],
                    in_=masks[:, o * 512 : (o + 1) * 512],
                    compare_op=mybir.AluOpType.is_ge,
                    fill=0.0,
                    base=-(128 * o),
                    pattern=[[1, 512]],
                    channel_multiplier=-1,
                )
            nc.vector.memset(vext_sb, 1.0)

            # input DMAs (wk+xT first: kT projection is the critical path)
            for c in range(NE):
                nc.sync.dma_start(out=wk_sb[:, c * HD : (c + 1) * HD], in_=wk_d[c])
                nc.sync.dma_start(out=xT_sb[:, c * T : (c + 1) * T], in_=xT_d[c])
            for c in range(NE):
                nc.sync.dma_start(
                    out=wq_sb[:, c * GS * HD : (c + 1) * GS * HD], in_=wq_d[c]
                )
                nc.sync.dma_start(out=wv_sb[:, c * HD : (c + 1) * HD], in_=wv_d[c])
            for c in range(GS):
                nc.sync.dma_start(out=wp_sb[:, c * EMB : (c + 1) * EMB], in_=wp_d[c])

            # kT projection: kT[d, t], contraction over EMB chunks
            for tp in range(4):
                ps = pmm.tile([128, 512], f32, tag="mm")
                for c in range(NE):
                    nc.tensor.matmul(
                        ps,
                        lhsT=wk_sb[:, c * HD : (c + 1) * HD],
                        rhs=xT_sb[:, c * T + tp * 512 : c * T + (tp + 1) * 512],
                        start=(c == 0),
                        stop=(c == NE - 1),
                    )
                nc.scalar.copy(kT_sb[:, tp * 512 : (tp + 1) * 512], ps)

            # qT projection: per head s
            for s in range(GS):
                for tp in range(4):
                    ps = pmm.tile([128, 512], f32, tag="mm")
                    for c in range(NE):
                        nc.tensor.matmul(
                            ps,
                            lhsT=wq_sb[
                                :, c * GS * HD + s * HD : c * GS * HD + (s + 1) * HD
                            ],
                            rhs=xT_sb[:, c * T + tp * 512 : c * T + (tp + 1) * 512],
                            start=(c == 0),
                            stop=(c == NE - 1),
                        )
                    nc.scalar.copy(
                        qT_sb[:, s * T + tp * 512 : s * T + (tp + 1) * 512], ps
                    )

            # v projection: v[t, d] (natural layout; stationary = xT chunk)
            for tt in range(NT):
                ps = pmm.tile([128, 128], f32, tag="mm")
                for c in range(NE):
                    nc.tensor.matmul(
                        ps,
                        lhsT=xT_sb[:, c * T + tt * 128 : c * T + (tt + 1) * 128],
                        rhs=wv_sb[:, c * HD : (c + 1) * HD],
                        start=(c == 0),
                        stop=(c == NE - 1),
                    )
                nc.vector.tensor_copy(
                    vext_sb[:, tt * (HD + 1) : tt * (HD + 1) + HD], ps
                )

            # attention + output projection, pipelined by q-pass
            for qp in range(NQP):
                for s in range(GS):
                    q_sl = slice(s * T + qp * 512, s * T + (qp + 1) * 512)
                    pts = []
                    for j in range(4 * qp + 4):
                        ps = pmm.tile([128, 512], f32, tag="mm")
                        nc.tensor.matmul(
                            ps,
                            lhsT=kT_sb[:, j * 128 : (j + 1) * 128],
                            rhs=qT_sb[:, q_sl],
                            start=True,
                            stop=True,
                        )
                        pt = ptp.tile([128, 512], bf16, tag="pt")
                        nc.scalar.activation(
                            pt, ps, mybir.ActivationFunctionType.Exp, scale=SCALE
                        )
                        o = j - 4 * qp
                        if o >= 0:
                            nc.vector.tensor_mul(
                                pt, pt, masks[:, o * 512 : (o + 1) * 512]
                            )
                        pts.append(pt)
                    for u in range(4):
                        jmax = 4 * qp + u
                        oe = pox.tile([128, HD + 1], f32, tag="oext")
                        for j in range(jmax + 1):
                            nc.tensor.matmul(
                                oe,
                                lhsT=pts[j][:, u * 128 : (u + 1) * 128],
                                rhs=vext_sb[:, j * (HD + 1) : (j + 1) * (HD + 1)],
                                start=(j == 0),
                                stop=(j == jmax),
                            )
                        rc = smp.tile([128, 1], f32, tag="rc")
                        nc.vector.reciprocal(rc, oe[:, HD : HD + 1])
                        on = onp.tile([128, 128], bf16, tag="on")
                        nc.vector.tensor_scalar_mul(on, oe[:, 0:HD], rc)
                        tps = ptr.tile([128, 128], bf16, tag="tr")
                        nc.tensor.transpose(tps, on, ident)
                        tq = qp * 512 + u * 128
                        nc.scalar.copy(ohT_sb[:, s * T + tq : s * T + tq + 128], tps)
                # output projection for this q-pass's 4 row tiles
                for u in range(4):
                    tt = qp * 4 + u
                    for jp in range(4):
                        ps = pmm.tile([128, 512], f32, tag="mm")
                        for s in range(GS):
                            nc.tensor.matmul(
                                ps,
                                lhsT=ohT_sb[:, s * T + tt * 128 : s * T + (tt + 1) * 128],
                                rhs=wp_sb[:, s * EMB + jp * 512 : s * EMB + (jp + 1) * 512],
                                start=(s == 0),
                                stop=(s == GS - 1),
                            )
                        ot = osp.tile([128, 512], f32, tag="ostage")
                        nc.vector.tensor_copy(ot, ps)
                        nc.sync.dma_start(
                            out=out_d[tt, :, jp * 512 : (jp + 1) * 512], in_=ot
                        )

    nc.finalize()
    return nc


def _get_program():
    global _PROGRAM
    if _PROGRAM is None:
        _PROGRAM = _build_program()
    return _PROGRAM


def _make_in_maps(x, Wq, Wk, Wv, Wp):
    in_maps = []
    xTs = [np.asarray(x[b]).T.astype(_BF16) for b in range(2)]
    for c in range(8):
        b, g = c // 4, c % 4
        sl = slice(g * GS * HD, (g + 1) * GS * HD)
        kv = slice(g * GS * HD, g * GS * HD + HD)
        in_maps.append(
            {
                "xT": xTs[b],
                "wqT": np.asarray(Wq[sl, :]).T.astype(_BF16),
                "wkT": np.asarray(Wk[kv, :]).T.astype(_BF16),
                "wvT": np.asarray(Wv[kv, :]).T.astype(_BF16),
                "wpT": np.asarray(Wp[:, sl]).T.astype(_BF16),
            }
        )
    return in_maps


def run(x, Wq, Wk, Wv, Wp, bp, trace=False, **trace_kwargs):
    from concourse.bass_utils import run_bass_kernel_spmd

    nc = _get_program()
    in_maps = _make_in_maps(x, Wq, Wk, Wv, Wp)
    res = run_bass_kernel_spmd(
        nc, in_maps, core_ids=list(range(8)), trace=trace, **trace_kwargs
    )
    bp = np.asarray(bp, dtype=np.float32)
    y = np.empty((2, T, EMB), dtype=np.float32)
    for b in range(2):
        acc = res.results[4 * b]["partial"].copy()
        for g in range(1, 4):
            acc += res.results[4 * b + g]["partial"]
        y[b] = acc + bp
    return y, res


def kernel(x, Wq, Wk, Wv, Wp, bp):
    y, _ = run(x, Wq, Wk, Wv, Wp, bp, trace=False)
    return y
